# revision 4
# baseline (speedup 1.0000x reference)
"""Trainium2 Bass kernel for nn_MultiHeadAttention (B=8, S=1024, D=1024, h=16).

Sharding: pure data-parallel over batch — each of the 8 NeuronCores computes
the full MHA for one batch element. No collectives.

Per-core pipeline (bf16 matmul operands, fp32 PSUM accumulation), restructured
as a software pipeline over head pairs so the ScalarE softmax-exp chain
(~18us/pair, the per-pair bottleneck) starts early and runs continuously:

  1. V/Q/K prep: load row-major, PE-transpose 128x128 blocks into
     feature-major bf16 tiles (one shared 16-slot ring: V uses slots then K
     reuses them after v_proj).
  2. v_proj first -> vaug tiles (row-major head-major Vp + a ones column per
     head that accumulates the softmax denominator inside the PV matmul).
  3. Per head-pair iteration d: qproj(d) -> qpt ring tile, kproj(d) -> kpt,
     scores(d) (row-split head pair: even head on PE rows 0-63, odd on
     64-127, concurrent in the array) + exp on ScalarE, then PV(d-1) whose
     exp finished during the previous iteration.
  4. Softmax division off a SBUF staging copy via a DRAM-spread reciprocal
     broadcast (keeps division off the PE/ACT critical path).
  5. o_proj: db=0..6 accumulation chains open before the last pair's
     division lands; db=7 contributions + relu evictions close them.
"""
import os
from contextlib import ExitStack

import numpy as np

import concourse.bass as bass
import concourse.tile as tile
from concourse import mybir
from concourse.bass_utils import run_bass_kernel_spmd
from concourse.masks import make_identity

f32 = mybir.dt.float32
bf16 = mybir.dt.bfloat16
AF = mybir.ActivationFunctionType
ALU = mybir.AluOpType

S = 1024
D = 1024
H = 16
DK = 64
P = 128
NB = D // P  # 8 blocks
QC = 512
N_CORES = 8


def _split_wide_waits(nc, max_waits=1):
    """This walrus build rejects instructions carrying more than one
    semaphore wait; move excess waits onto NoOp carriers inserted before
    the offending instruction on the same engine."""
    for bb in nc.m.functions[0].blocks:
        idx = 0
        while idx < len(bb.instructions):
            ins = bb.instructions[idx]
            si = ins.sync_info
            if si is not None and si.on_wait and len(si.on_wait) > max_waits:
                waits = list(si.on_wait)
                rest, keep = waits[:-max_waits], waits[-max_waits:]
                for j in range(0, len(rest), max_waits):
                    nop = mybir.InstNoOp(
                        name=f"I-waitsplit-{nc.next_id()}",
                        engine=ins.engine,
                        ins=[],
                        outs=[],
                    )
                    nop.sync_info = mybir.SyncInfo(
                        on_wait=rest[j : j + max_waits], on_update=[]
                    )
                    nc.register_instruction(nop)
                    bb.instructions.insert(idx, nop)
                    idx += 1
                ins.sync_info = mybir.SyncInfo(
                    on_wait=keep, on_update=list(si.on_update)
                )
            idx += 1


def _build_nc(with_bv: bool, with_bo: bool):
    nc = bass.Bass("TRN2", target_bir_lowering=False, debug=False, num_devices=1)

    Qd = nc.dram_tensor("Q", [S, D], f32, kind="ExternalInput").ap()
    Kd = nc.dram_tensor("K", [S, D], f32, kind="ExternalInput").ap()
    Vd = nc.dram_tensor("V", [S, D], f32, kind="ExternalInput").ap()
    WQd = nc.dram_tensor("WQ", [D, D], f32, kind="ExternalInput").ap()
    WKd = nc.dram_tensor("WK", [D, D], f32, kind="ExternalInput").ap()
    WVd = nc.dram_tensor("WV", [D, D], f32, kind="ExternalInput").ap()
    WOd = nc.dram_tensor("WO", [D, D], f32, kind="ExternalInput").ap()
    bQd = nc.dram_tensor("bQ", [D], f32, kind="ExternalInput").ap()
    bKd = nc.dram_tensor("bK", [D], f32, kind="ExternalInput").ap()
    bVd = nc.dram_tensor("bV", [D], f32, kind="ExternalInput").ap()
    bOd = nc.dram_tensor("bO", [D], f32, kind="ExternalInput").ap()
    outd = nc.dram_tensor("out", [S, D], f32, kind="ExternalOutput").ap()

    with tile.TileContext(nc) as tc, ExitStack() as ctx:
        sb = ctx.enter_context(tc.tile_pool(name="sb", bufs=1))
        ps = ctx.enter_context(tc.tile_pool(name="ps", bufs=1, space="PSUM"))
        dramp = ctx.enter_context(tc.tile_pool(name="dram", bufs=1, space="DRAM"))

        # ---- constants -------------------------------------------------
        ident = sb.tile([P, P], f32, tag="ident", name="ident")
        make_identity(nc, ident)
        identb = sb.tile([P, P], bf16, tag="identb", name="identb")
        nc.vector.tensor_copy(identb, ident)
        bqk = sb.tile([P, 2 * NB], f32, tag="bqk", name="bqk")
        nc.sync.dma_start(bqk[:, 0:NB], bQd.rearrange("(db p) -> p db", p=P))
        nc.sync.dma_start(bqk[:, NB : 2 * NB], bKd.rearrange("(db p) -> p db", p=P))
        if with_bv:
            bvb = sb.tile([P, D], f32, tag="bvb", name="bvb")
            nc.sync.dma_start(bvb, bVd[None, :].broadcast_to([P, D]))
        if with_bo:
            bob = sb.tile([P, D], f32, tag="bob", name="bob")
            nc.sync.dma_start(bob, bOd[None, :].broadcast_to([P, D]))

        def wload(Wd, kb, chunk, tag="wbf", bufs=18):
            """Stream a [128, 512] f32 weight strip and cast to bf16."""
            wstage = sb.tile([P, QC], f32, tag="wstage", bufs=3, name="wstage")
            nc.sync.dma_start(
                wstage, Wd[kb * P : (kb + 1) * P, chunk * QC : (chunk + 1) * QC]
            )
            wb = sb.tile([P, QC], bf16, tag=tag, bufs=bufs, name=tag)
            nc.vector.tensor_copy(wb, wstage)
            return wb

        def load_transpose(Xd, slot0):
            """HBM row-major -> feature-major bf16 tiles xt[db] (128 x 1024).
            All three tensors share one 16-slot ring; V uses slots 0-7 first,
            Q takes 8-15, K reuses 0-7 once v_proj has consumed them."""
            xt = [
                sb.tile([P, S], bf16, tag="xt", bufs=16, name=f"xt{slot0 + i}")
                for i in range(NB)
            ]
            for sblk in range(NB):
                xn = sb.tile([P, D], f32, tag="xn", bufs=2, name="xn")
                nc.sync.dma_start(xn, Xd[sblk * P : (sblk + 1) * P, :])
                xnb = sb.tile([P, D], bf16, tag="xnb", bufs=2, name="xnb")
                nc.vector.tensor_copy(xnb, xn)
                for db in range(NB):
                    tp = ps.tile([P, 2, QC], f32, tag="big", bufs=3, name="tp")
                    tpb = tp[:, 0, 0:P].bitcast(bf16)[:, 0:P]
                    nc.tensor.transpose(
                        tpb, xnb[:, db * P : (db + 1) * P], identb
                    )
                    dst = xt[db][:, sblk * P : (sblk + 1) * P]
                    if (sblk + db) % 2 == 0:
                        nc.vector.tensor_copy(dst, tpb)
                    else:
                        nc.scalar.activation(dst, tpb, AF.Copy)
            return xt

        # ---- preps: V first (v_proj consumes vt before k_prep reuses the
        # ring slots — k_prep MUST be emitted after v_proj or the PE queue
        # deadlocks: k-transposes would wait on psum freed by evictions that
        # wait on xt slots only v_proj's later matmuls release) ------------
        with nc.named_scope("v_prep"):
            vt = load_transpose(Vd, 0)
        with nc.named_scope("q_prep"):
            qt = load_transpose(Qd, 8)

        # ---- V projection -> vaug (row-major, head-major, 65th=ones) ----
        with nc.named_scope("v_proj"):
            vaug = [
                sb.tile([P, H * 65], bf16, tag="vaug", bufs=NB, name=f"vaug{i}")
                for i in range(NB)
            ]
            for sblk in range(NB):
                nc.vector.memset(
                    vaug[sblk].rearrange("p (h c) -> p h c", c=65)[:, :, 64:65],
                    1.0,
                )
            wv = [[wload(WVd, kb, c) for c in range(2)] for kb in range(NB)]
            for sblk in range(NB):
                acc = [
                    ps.tile([P, QC], f32, tag="vp", bufs=2, name="vacc")
                    for _ in range(2)
                ]
                for kb in range(NB):
                    for c in range(2):
                        nc.tensor.matmul(
                            acc[c],
                            vt[kb][:, sblk * P : (sblk + 1) * P],
                            wv[kb][c],
                            start=(kb == 0),
                            stop=(kb == NB - 1),
                        )
                for c in range(2):
                    if with_bv:
                        nc.vector.tensor_add(
                            acc[c], acc[c], bvb[:, c * QC : (c + 1) * QC]
                        )
                    dst = vaug[sblk].rearrange("p (h c) -> p h c", c=65)[
                        :, c * 8 : (c + 1) * 8, 0:64
                    ]
                    nc.scalar.activation(
                        dst, acc[c].rearrange("p (h c) -> p h c", c=64), AF.Relu
                    )

        # ---- per-pair pipeline helpers ----------------------------------
        def projd(xt, wstrips, d, bias_base, tag):
            """One projection output block: relu(W[:, dblk].T @ X^T + b)
            evicted to a feature-major bf16 ring tile on VectorE."""
            acc = ps.tile([P, 2, QC], f32, tag="big", bufs=3, name="pacc")
            co = (d % 4) * P
            for kb in range(NB):
                wt = wstrips[kb][:, co : co + P]
                first, last = kb == 0, kb == NB - 1
                nc.tensor.matmul(
                    acc[:, 0, :], wt, xt[kb][:, 0:QC], start=first, stop=last
                )
                nc.tensor.matmul(
                    acc[:, 1, :], wt, xt[kb][:, QC:S], start=first, stop=last
                )
            xpt = sb.tile([P, S], bf16, tag=tag, bufs=2, name=tag)
            nc.vector.tensor_scalar(
                out=xpt.rearrange("p (c q) -> p c q", c=2),
                in0=acc,
                scalar1=bqk[:, bias_base + d : bias_base + d + 1],
                scalar2=0.0,
                op0=ALU.add,
                op1=ALU.max,
            )
            return xpt

        def emit_scores_unit(d, qpt, kpt):
            """Scores + exp for head pair d. Even head on PE rows 0-63, odd
            on 64-127: the B matmuls ride concurrently in the array."""
            ptA = sb.tile([P, NB, 2, QC], bf16, tag="pt", bufs=4, name="ptA")
            ptB = sb.tile([P, NB, 2, QC], bf16, tag="pt", bufs=4, name="ptB")
            for kb in range(NB):
                ksl = slice(kb * P, (kb + 1) * P)
                spA = ps.tile([P, 2, QC], f32, tag="big", bufs=3, name="spA")
                spB = ps.tile([P, 2, QC], f32, tag="big", bufs=3, name="spB")
                for qc in range(2):
                    qsl = slice(qc * QC, (qc + 1) * QC)
                    nc.tensor.matmul(
                        spA[:, qc, :], kpt[0:DK, ksl], qpt[0:DK, qsl],
                        start=True, stop=True,
                    )
                for qc in range(2):
                    qsl = slice(qc * QC, (qc + 1) * QC)
                    nc.tensor.matmul(
                        spB[:, qc, :], kpt[DK:P, ksl], qpt[DK:P, qsl],
                        start=True, stop=True,
                    )
                nc.scalar.activation(ptA[:, kb, :, :], spA, AF.Exp, scale=0.03125)
                nc.scalar.activation(ptB[:, kb, :, :], spB, AF.Exp, scale=0.03125)
            return ptA, ptB

        ot = [
            sb.tile([P, S], bf16, tag="ot", bufs=NB, name=f"ot{i}")
            for i in range(NB)
        ]

        def emit_pv_tail(h, vp):
            """Softmax division: reciprocal of the denominator row on a
            DRAM-spread layout + DMA broadcast + multiply (all off PE/ACT)."""
            dbq, off = h // 2, (h % 2) * DK
            for qc in range(2):
                qsl = slice(qc * QC, (qc + 1) * QC)
                stage = sb.tile([65, QC], f32, tag="stage", bufs=3, name="stage")
                nc.vector.tensor_copy(stage, vp[qc][0:65, :])
                scr = dramp.tile([1, QC], f32, tag="scr", bufs=6, name="scr")
                nc.gpsimd.dma_start(scr, stage[64:65, :])
                rcp = sb.tile([DK, NB], f32, tag="rcp", bufs=3, name="rcp")
                nc.gpsimd.dma_start(
                    rcp, scr.rearrange("o (a b) -> a (o b)", a=DK)
                )
                nc.vector.reciprocal(rcp, rcp)
                scr2 = dramp.tile([1, QC], f32, tag="scr2", bufs=6, name="scr2")
                nc.gpsimd.dma_start(
                    scr2.rearrange("o (a b) -> a (o b)", a=DK), rcp
                )
                bc = sb.tile([DK, QC], f32, tag="bc", bufs=2, name="bc")
                nc.gpsimd.dma_start(bc, scr2.broadcast_to([DK, QC]))
                if off == 0:
                    nc.vector.tensor_mul(ot[dbq][0:DK, qsl], stage[0:DK, :], bc)
                else:
                    tmp = sb.tile([DK, QC], bf16, tag="tmp", bufs=2, name="tmp")
                    nc.vector.tensor_mul(tmp, stage[0:DK, :], bc)
                    nc.gpsimd.dma_start(ot[dbq][DK:P, qsl], tmp)

        def emit_pv_pair(d, ptA, ptB):
            """PV + division for head pair (2d, 2d+1), both q-chunks."""
            for hl, ptX in ((0, ptA), (1, ptB)):
                h = 2 * d + hl
                vp = [
                    ps.tile([P, QC], f32, tag="vp", bufs=2, name="vpacc")
                    for _ in range(2)
                ]
                for kb in range(NB):
                    for qc in range(2):
                        nc.tensor.matmul(
                            vp[qc][0:65, :],
                            vaug[kb][:, h * 65 : (h + 1) * 65],
                            ptX[:, kb, qc, :],
                            start=(kb == 0),
                            stop=(kb == NB - 1),
                        )
                emit_pv_tail(h, vp)

        # ---- attention pipeline over head pairs -------------------------
        wq = wk = None
        pend = None
        for d in range(NB):
            with nc.named_scope(f"it{d}"):
                if d % 4 == 0:
                    half = d // 4
                    wq = [wload(WQd, kb, half) for kb in range(NB)]
                    wk = [wload(WKd, kb, half) for kb in range(NB)]
                qpt = projd(qt, wq, d, 0, "qpt")
                kpt = projd(kt, wk, d, NB, "kpt")
                cur = emit_scores_unit(d, qpt, kpt)
                if d == 6:
                    wo = [[wload(WOd, db, c, tag="wo", bufs=16) for c in range(2)]
                          for db in range(NB)]
                if pend is not None:
                    emit_pv_pair(d - 1, *pend)
                pend = cur

        # ---- tail: open o_proj chains for db 0-6, then last PV ----------
        def oproj_open(sblk):
            bigacc = ps.tile([P, 2, QC], f32, tag="big", bufs=3, name="oacc")
            for db in range(NB - 1):
                for c in range(2):
                    nc.tensor.matmul(
                        bigacc[:, c, :],
                        ot[db][:, sblk * P : (sblk + 1) * P],
                        wo[db][c],
                        start=(db == 0),
                        stop=False,
                    )
            return bigacc

        def oproj_close(sblk, bigacc):
            for c in range(2):
                nc.tensor.matmul(
                    bigacc[:, c, :],
                    ot[NB - 1][:, sblk * P : (sblk + 1) * P],
                    wo[NB - 1][c],
                    start=False,
                    stop=True,
                )
            for c in range(2):
                if with_bo:
                    nc.vector.tensor_add(
                        bigacc[:, c, :], bigacc[:, c, :], bob[:, c * QC : (c + 1) * QC]
                    )
                o = sb.tile([P, QC], f32, tag="obuf", bufs=2, name="obuf")
                nc.scalar.activation(o, bigacc[:, c, :], AF.Relu)
                nc.sync.dma_start(
                    outd[sblk * P : (sblk + 1) * P, c * QC : (c + 1) * QC], o
                )

        with nc.named_scope("o_proj"):
            chains = {}
            for sblk in range(2):
                chains[sblk] = oproj_open(sblk)
            with nc.named_scope("pv_last"):
                emit_pv_pair(NB - 1, *pend)
            for sblk in range(2, NB):
                chains[sblk] = oproj_open(sblk)
                oproj_close(sblk - 2, chains.pop(sblk - 2))
            for sblk in (NB - 2, NB - 1):
                oproj_close(sblk, chains.pop(sblk))

    _split_wide_waits(nc)
    return nc


_NC_CACHE = {}


def kernel(Q, K, V, WQ, bQ, WK, bK, WV, bV, WO, bO, h):
    Q, K, V = (np.ascontiguousarray(np.asarray(x, np.float32)) for x in (Q, K, V))
    WQ, WK, WV, WO = (
        np.ascontiguousarray(np.asarray(x, np.float32)) for x in (WQ, WK, WV, WO)
    )
    bQ, bK, bV, bO = (
        np.ascontiguousarray(np.asarray(x, np.float32)) for x in (bQ, bK, bV, bO)
    )
    h = int(np.asarray(h))
    assert h == H, f"kernel specialized for h=16, got {h}"
    B = Q.shape[0]
    assert Q.shape == (B, S, D) and B == N_CORES

    key = (bool(np.any(bV)), bool(np.any(bO)))
    if key not in _NC_CACHE:
        _NC_CACHE[key] = _build_nc(*key)
    nc = _NC_CACHE[key]

    in_maps = [
        {
            "Q": Q[b], "K": K[b], "V": V[b],
            "WQ": WQ, "WK": WK, "WV": WV, "WO": WO,
            "bQ": bQ, "bK": bK, "bV": bV, "bO": bO,
        }
        for b in range(B)
    ]
    trace = os.environ.get("BASS_MHA_TRACE") == "1"
    res = run_bass_kernel_spmd(
        nc, in_maps, core_ids=list(range(N_CORES)), trace=trace
    )
    if trace:
        kernel.last_results = res
    return np.stack([res.results[b]["out"] for b in range(B)], axis=0)
